# revision 6
# baseline (speedup 1.0000x reference)
"""GroupedQueryAttention TRN2 kernel — 8-core tensor-parallel.

Reference problem: B=2, S=2048, D=2048, H=16 q-heads, HKV=2 kv-heads, DH=128.
Sharding: core c handles batch b=c//4 and q-head group qg=c%4 (4 consecutive
q-heads = rows [qg*512,(qg+1)*512) of Wq), kv head qg//2. Each core computes a
partial output y_c = attn_out_c @ Wo[:, cols_c].T of shape [S, D] (stored
transposed as yT [D, S]); the host sums the 4 partials per batch and adds bo.

Device-side layout: everything "feature-major" (transposed). Host passes
xT=x[b].T, and pre-transposed weight slices, so no transposes of x/W on device.
Scores are computed transposed, ST[sk, sq] = K @ Q.T, so that exp(ST) feeds the
attn@V matmul directly as the moving operand (lhsT = V tile in natural [sk,dh]
layout). Softmax denominators come from a ones-vector matmul accumulated along
with AV; normalization is applied to the small OT[dh,sq] accumulator via a
DMA-broadcast reciprocal row. Causal masking: block-skipping on sk tiles plus a
[128,128] triangular mask multiply on diagonal blocks. All matmuls run as
float32r (full PE rate at moving-dim >= 256, ~TF32 precision).
"""

import os
import sys
from contextlib import ExitStack

import numpy as np

sys.path.insert(0, "/opt/trn_rl_repo")
sys.path.insert(0, "/opt/pypackages")

import concourse.bass as bass  # noqa: E402
import concourse.mybir as mybir  # noqa: E402
import concourse.tile as tile  # noqa: E402
from concourse import bacc  # noqa: E402
from concourse import bass_utils  # noqa: E402
from concourse.masks import make_identity  # noqa: E402

B, S, D = 2, 2048, 2048
H, HKV, DH = 16, 2, 128
NCORES = 8
NH = 4          # q heads per core
QD = NH * DH    # 512 q-dims per core
P = 128
SB = 512        # seq block width
NSB = S // SB   # 4 seq blocks
ND = D // P     # 16 d tiles
NS = S // P     # 16 seq tiles
SCALE = float(1.0 / np.sqrt(DH))

F32 = mybir.dt.float32
R = mybir.dt.float32r
EXP = mybir.ActivationFunctionType.Exp


def _body(ctx, tc, nc, io):
    sing = ctx.enter_context(tc.tile_pool(name="sing", bufs=1))
    xtp = ctx.enter_context(tc.tile_pool(name="xtp", bufs=17))
    qtp = ctx.enter_context(tc.tile_pool(name="qtp", bufs=8))
    ptp = ctx.enter_context(tc.tile_pool(name="ptp", bufs=6))
    otp = ctx.enter_context(tc.tile_pool(name="otp", bufs=8))
    smal = ctx.enter_context(tc.tile_pool(name="smal", bufs=3))
    drp = ctx.enter_context(tc.tile_pool(name="drp", bufs=2, space="DRAM"))
    ps_big = ctx.enter_context(tc.tile_pool(name="ps_big", bufs=2, space="PSUM"))
    ps_st = ctx.enter_context(tc.tile_pool(name="ps_st", bufs=4, space="PSUM"))
    ps_ot = ctx.enter_context(tc.tile_pool(name="ps_ot", bufs=2, space="PSUM"))

    # ---- resident weights / constants ----
    wq_sb = sing.tile([P, ND, QD], R, name="wq_sb")
    nc.sync.dma_start(wq_sb, io["wq"].rearrange("(n p) q -> p n q", p=P))
    wk_sb = sing.tile([P, ND, DH], R, name="wk_sb")
    nc.sync.dma_start(wk_sb, io["wk"].rearrange("(n p) q -> p n q", p=P))
    wv_sb = sing.tile([P, ND, DH], R, name="wv_sb")
    nc.sync.dma_start(wv_sb, io["wv"].rearrange("(n p) q -> p n q", p=P))
    wo_sb = sing.tile([P, NH, D], R, name="wo_sb")
    nc.sync.dma_start(wo_sb, io["wo"].rearrange("(n p) q -> p n q", p=P))
    bq_sb = sing.tile([P, NH], F32, name="bq_sb")
    nc.sync.dma_start(bq_sb, io["bq"].rearrange("(h p) -> p h", p=P))
    bk_sb = sing.tile([P, 1], F32, name="bk_sb")
    nc.sync.dma_start(bk_sb, io["bk"].rearrange("(o p) -> p o", p=P))
    bv_sb = sing.tile([P, 1], F32, name="bv_sb")
    nc.sync.dma_start(bv_sb, io["bv"].rearrange("(o p) -> p o", p=P))
    msk_sb = sing.tile([P, P], R, name="msk_sb")
    nc.sync.dma_start(msk_sb, io["msk"])
    ones_sb = sing.tile([P, 1], R, name="ones_sb")
    nc.sync.dma_start(ones_sb, io["ones1"])
    ident = sing.tile([P, P], F32, name="ident")
    make_identity(nc, ident)

    kt_sb = sing.tile([P, S], R, name="kt_sb")   # [dh, sk]
    v_sb = sing.tile([P, S], R, name="v_sb")     # [sk_local, t*128+dh]

    for j in range(NSB):
        # ---- load x tiles for this seq block ----
        xts = []
        for d in range(ND):
            xt = xtp.tile([P, SB], R, name=f"xt_{j}_{d}", tag="xt")
            nc.sync.dma_start(xt, io["xT"][d * P:(d + 1) * P, j * SB:(j + 1) * SB])
            xts.append(xt)

        # ---- K projection: KT[dh, sk-block j] ----
        psk = ps_big.tile([P, SB], F32, name=f"psk_{j}", tag="big")
        for d in range(ND):
            nc.tensor.matmul(psk, wk_sb[:, d, :], xts[d],
                             start=(d == 0), stop=(d == ND - 1))
        nc.vector.tensor_scalar_add(kt_sb[:, j * SB:(j + 1) * SB], psk, bk_sb[:, 0:1])

        # ---- V projection (transposed), then PE-transpose to natural ----
        psv = ps_big.tile([P, SB], F32, name=f"psv_{j}", tag="big")
        for d in range(ND):
            nc.tensor.matmul(psv, wv_sb[:, d, :], xts[d],
                             start=(d == 0), stop=(d == ND - 1))
        vt_tmp = smal.tile([P, SB], F32, name=f"vt_{j}", tag="vt")
        nc.vector.tensor_scalar_add(vt_tmp, psv, bv_sb[:, 0:1])
        for c in range(SB // P):
            pst = ps_big.tile([P, P], F32, name=f"psvt_{j}_{c}", tag="big")
            nc.tensor.transpose(pst, vt_tmp[:, c * P:(c + 1) * P], ident)
            t_glob = 4 * j + c
            nc.vector.tensor_copy(v_sb[:, t_glob * P:(t_glob + 1) * P], pst)

        # ---- Q projections for 4 heads ----
        qts = []
        for h in range(NH):
            psq = ps_big.tile([P, SB], F32, name=f"psq_{j}_{h}", tag="big")
            for d in range(ND):
                nc.tensor.matmul(psq, wq_sb[:, d, h * P:(h + 1) * P],
                                 xts[d],
                                 start=(d == 0), stop=(d == ND - 1))
            qt = qtp.tile([P, SB], R, name=f"qt_{j}_{h}", tag="qt")
            nc.vector.tensor_scalar_add(qt, psq, bq_sb[:, h:h + 1])
            qts.append(qt)

        # ---- attention per head ----
        ots = []
        nt = 4 * j + 4  # number of causal sk tiles for this sq block
        for h in range(NH):
            ot_ps = ps_ot.tile([P, SB], F32, name=f"ot_{j}_{h}", tag="ot")
            den_ps = ps_st.tile([1, SB], F32, name=f"den_{j}_{h}", tag="st")
            for t in range(nt):
                off = P * max(0, t - 4 * j)
                st = ps_st.tile([P, SB], F32, name=f"st_{j}_{h}_{t}", tag="st")
                nc.tensor.matmul(st[:, off:], kt_sb[:, t * P:(t + 1) * P],
                                 qts[h][:, off:], start=True, stop=True)
                pt = ptp.tile([P, SB], R, name=f"pt_{j}_{h}_{t}", tag="pt")
                nc.scalar.activation(pt[:, off:], st[:, off:], EXP, scale=SCALE)
                if t >= 4 * j:
                    c = t - 4 * j
                    nc.vector.tensor_mul(pt[:, c * P:(c + 1) * P],
                                         pt[:, c * P:(c + 1) * P], msk_sb)
                nc.tensor.matmul(ot_ps[:, off:], v_sb[:, t * P:(t + 1) * P],
                                 pt[:, off:],
                                 start=(t == 0), stop=(t == nt - 1))
                nc.tensor.matmul(den_ps[:, off:], ones_sb,
                                 pt[:, off:],
                                 start=(t == 0), stop=(t == nt - 1))
            rec = smal.tile([1, SB], F32, name=f"rec_{j}_{h}", tag="rec")
            nc.vector.reciprocal(rec, den_ps)
            rdr = drp.tile([1, SB], F32, name=f"rdr_{j}_{h}", tag="rdr")
            nc.sync.dma_start(rdr, rec)
            bc = smal.tile([P, SB], F32, name=f"bc_{j}_{h}", tag="bc")
            nc.sync.dma_start(bc, bass.AP(
                tensor=rdr.tensor, offset=rdr.offset,
                ap=[[0, P]] + list(rdr.ap[1:])))
            ot = otp.tile([P, SB], R, name=f"otn_{j}_{h}", tag="ot_sb")
            nc.vector.tensor_mul(ot, ot_ps, bc)
            ots.append(ot)

        # ---- output projection for this seq block ----
        for dd in range(ND):
            yps = ps_big.tile([P, SB], F32, name=f"yps_{j}_{dd}", tag="big")
            for h in range(NH):
                nc.tensor.matmul(yps, wo_sb[:, h, dd * P:(dd + 1) * P],
                                 ots[h], start=(h == 0), stop=(h == NH - 1))
            yt_sb = ptp.tile([P, SB], F32, name=f"ytsb_{j}_{dd}", tag="yt")
            nc.vector.tensor_copy(yt_sb, yps)
            nc.sync.dma_start(io["yT"][dd * P:(dd + 1) * P, j * SB:(j + 1) * SB], yt_sb)


def build():
    nc = bacc.Bacc("TRN2", target_bir_lowering=False, debug=False,
                   num_devices=NCORES)
    io = {}
    io["xT"] = nc.dram_tensor("xT", [D, S], R, kind="ExternalInput").ap()
    io["wq"] = nc.dram_tensor("wq", [D, QD], R, kind="ExternalInput").ap()
    io["wk"] = nc.dram_tensor("wk", [D, DH], R, kind="ExternalInput").ap()
    io["wv"] = nc.dram_tensor("wv", [D, DH], R, kind="ExternalInput").ap()
    io["wo"] = nc.dram_tensor("wo", [QD, D], R, kind="ExternalInput").ap()
    io["bq"] = nc.dram_tensor("bq", [QD], F32, kind="ExternalInput").ap()
    io["bk"] = nc.dram_tensor("bk", [DH], F32, kind="ExternalInput").ap()
    io["bv"] = nc.dram_tensor("bv", [DH], F32, kind="ExternalInput").ap()
    io["msk"] = nc.dram_tensor("msk", [P, P], R, kind="ExternalInput").ap()
    io["ones1"] = nc.dram_tensor("ones1", [P, 1], R, kind="ExternalInput").ap()
    io["yT"] = nc.dram_tensor("yT", [D, S], F32, kind="ExternalOutput").ap()
    with tile.TileContext(nc) as tc, ExitStack() as ctx:
        _body(ctx, tc, nc, io)
    nc.compile()
    return nc


def make_in_maps(x, Wq, bq, Wk, bk, Wv, bv, Wo, bo):
    x = np.asarray(x, np.float32)
    xT = [np.ascontiguousarray(x[b].T) for b in range(B)]
    msk = np.triu(np.ones((P, P), np.float32))  # keep p <= f
    in_maps = []
    for c in range(NCORES):
        b, qg = c // 4, c % 4
        kv = qg // 2
        in_maps.append({
            "xT": xT[b],
            "wq": np.ascontiguousarray(np.asarray(Wq)[qg * QD:(qg + 1) * QD, :].T),
            "wk": np.ascontiguousarray(np.asarray(Wk)[kv * DH:(kv + 1) * DH, :].T),
            "wv": np.ascontiguousarray(np.asarray(Wv)[kv * DH:(kv + 1) * DH, :].T),
            "wo": np.ascontiguousarray(np.asarray(Wo)[:, qg * QD:(qg + 1) * QD].T),
            "bq": np.ascontiguousarray(np.asarray(bq)[qg * QD:(qg + 1) * QD]),
            "bk": np.ascontiguousarray(np.asarray(bk)[kv * DH:(kv + 1) * DH]),
            "bv": np.ascontiguousarray(np.asarray(bv)[kv * DH:(kv + 1) * DH]),
            "msk": msk,
            "ones1": np.ones((P, 1), np.float32),
        })
    return in_maps


_NC = None


def _get_nc():
    global _NC
    if _NC is None:
        _NC = build()
    return _NC


def kernel(x, Wq, bq, Wk, bk, Wv, bv, Wo, bo, _trace=False, **trace_kwargs):
    nc = _get_nc()
    in_maps = make_in_maps(x, Wq, bq, Wk, bk, Wv, bv, Wo, bo)
    res = bass_utils.run_bass_kernel_spmd(
        nc, in_maps, core_ids=list(range(NCORES)), trace=_trace, **trace_kwargs)
    y = np.zeros((B, S, D), np.float32)
    for c in range(NCORES):
        y[c // 4] += res.results[c]["yT"].T
    y += np.asarray(bo, np.float32)[None, None, :]
    if _trace:
        kernel.last_result = res
    return y
